# revision 1
# baseline (speedup 1.0000x reference)
"""PiLoraLayer TRN2 kernel: y = x + (alpha/r) * sin((2/pi) * (x @ A) @ B).

x: [4, 4096, 4096] f32; A = A_int8 * scale_A (per-col), B = B_int8 * scale_B
(per-col); rank 16 bottleneck.

Strategy (data-parallel over 8 NeuronCores):
- Host: dequantize the tiny weights once. Fold scale_A, scale_B and 1/pi^2
  into Bp = scale_A[:,None] * B_q * scale_B[None,:] / pi^2; keep A_q as f32.
  Then u = (x @ A_q) @ Bp equals arg/(2*pi) where arg = (2/pi)*h2, and
  y = x + 2*sin(2*pi*u).
- Shard x's 16384 token rows into 8 x [2048, 4096] shards, one per core.
- Device (per core), per 512-token super-tile:
    - DMA x in (4 chunks of [128, 4096]).
    - PE-transpose x into [128h, 512t] slabs; ACT copies PSUM->SBUF.
    - mm1: h1T[16, 512] = sum_k A_k.T @ xT_k (PSUM accumulate, 32 chunks)
    - mm2: per 128-token chunk, u_psum[128, 1024] = h1_c @ Bp_n (2-bank tile)
    - Range reduction (HW Sin LUT only accepts [-pi, pi]):
      k = (u + 1.5*2^23) - 1.5*2^23 in ONE two-op DVE tensor_scalar (RNE
      round-to-integer, written as bf16 which is exact for |k| <= 256);
      PE accumulates -k into the u bank via a bf16 negative-identity matmul,
      leaving frac in [-0.5, 0.5]; ACT computes s = sin(2*pi*frac) -> bf16.
    - DVE: s *= 2 (bf16 4x mode, in place), x_sb += s (mixed f32+bf16),
      DMA x_sb out as y.
- GPSIMD is kept out of the steady-state loop entirely: it is ~10x slower
  than DVE for elementwise work and its SBUF port sharing starves DVE.
"""

import sys

sys.path.insert(0, "/opt/trn_rl_repo")

import numpy as np

import concourse.bacc as bacc
import concourse.bass as bass
import concourse.tile as tile
from concourse import mybir
from concourse.bass import ts
from concourse.bass_utils import run_bass_kernel_spmd

P = 128
HIDDEN = 4096
RANK = 16
N_CORES = 8
TOTAL_ROWS = 4 * 4096
ROWS = TOTAL_ROWS // N_CORES  # 2048 per core
SUPER = 512  # tokens per steady-state super-tile
NCH = SUPER // P  # token chunks per super-tile
KC = HIDDEN // P  # 32 hidden chunks
UBLK = 1024  # tail block width (2 PSUM banks)
NUB = HIDDEN // UBLK  # 4 tail blocks per token chunk
ALPHA_OVER_R = 2.0  # 32.0 / 16
MAGIC = 12582912.0  # 1.5 * 2^23: f32 add/sub rounds to nearest integer
SCALE_2PI = 6.283185  # slightly < 2*pi so the LUT arg stays inside [-pi, pi]

F32 = mybir.dt.float32
F32R = mybir.dt.float32r  # replicated fp32: 1 cycle/row on PE when N>=256
BF16 = mybir.dt.bfloat16


def build_nc(rows: int = ROWS):
    """Build the per-core Bass program for a [rows, 4096] token shard."""
    assert rows % SUPER == 0
    n_super = rows // SUPER

    nc = bacc.Bacc(
        "TRN2",
        target_bir_lowering=False,
        debug=False,
        enable_asserts=False,
        num_devices=N_CORES,
    )
    x_d = nc.dram_tensor("x", [rows, HIDDEN], F32, kind="ExternalInput").ap()
    a_d = nc.dram_tensor("A", [HIDDEN, RANK], F32, kind="ExternalInput").ap()
    bp_d = nc.dram_tensor("Bp", [RANK, HIDDEN], F32, kind="ExternalInput").ap()
    i_d = nc.dram_tensor("I", [P, P], F32, kind="ExternalInput").ap()
    y_d = nc.dram_tensor("out", [rows, HIDDEN], F32, kind="ExternalOutput").ap()

    with tile.TileContext(nc) as tc:
        with (
            tc.tile_pool(name="singles", bufs=1) as singles,
            tc.tile_pool(name="xp", bufs=2) as xpool,
            tc.tile_pool(name="xtp", bufs=6) as xtpool,
            tc.tile_pool(name="kp", bufs=4) as kpool,
            tc.tile_pool(name="sp", bufs=4) as spool,
            tc.tile_pool(name="h1sb", bufs=2) as h1pool,
            tc.tile_pool(name="ptp", bufs=2, space="PSUM") as pt_psum,
            tc.tile_pool(name="h1p", bufs=2, space="PSUM") as h1_psum,
            tc.tile_pool(name="up", bufs=2, space="PSUM") as u_psum,
        ):
            ident = singles.tile([P, P], F32R)
            nc.sync.dma_start(out=ident[:], in_=i_d[:, :].bitcast(F32R))
            nident_bf = singles.tile([P, P], BF16)
            nc.gpsimd.memset(nident_bf[:], 0.0)
            nc.gpsimd.affine_select(
                out=nident_bf[:],
                in_=nident_bf[:],
                compare_op=mybir.AluOpType.not_equal,
                fill=-1.0,
                base=0,
                pattern=[[-1, P]],
                channel_multiplier=1,
            )
            a_sb = singles.tile([P, KC, RANK], F32R)
            nc.sync.dma_start(
                out=a_sb[:],
                in_=a_d.rearrange("(k p) r -> p k r", p=P).bitcast(F32R),
            )
            bp_sb = singles.tile([RANK, HIDDEN], F32R)
            nc.sync.dma_start(out=bp_sb[:], in_=bp_d[:, :].bitcast(F32R))

            def emit_tail_block(state, j):
                """One 1024-wide tail block j for a finished super-tile."""
                x_sb, h1_sb, row0, _nch = state
                c, nb = j // NUB, j % NUB
                u_ps = u_psum.tile([P, UBLK], F32)
                for jj in range(2):
                    nc.tensor.matmul(
                        u_ps[:, ts(jj, 512)],
                        h1_sb[:, ts(c, P)],
                        bp_sb[:, nb * UBLK + jj * 512 : nb * UBLK + (jj + 1) * 512],
                        start=True,
                        stop=True,
                    )
                kq = kpool.tile([P, UBLK], BF16)
                nc.vector.tensor_scalar(
                    kq[:],
                    u_ps[:],
                    MAGIC,
                    MAGIC,
                    mybir.AluOpType.add,
                    mybir.AluOpType.subtract,
                )
                for jj in range(2):
                    nc.tensor.matmul(
                        u_ps[:, ts(jj, 512)],
                        nident_bf[:],
                        kq[:, ts(jj, 512)],
                        start=False,
                        stop=True,
                        skip_group_check=True,
                    )
                s = spool.tile([P, UBLK], BF16)
                nc.scalar.activation(
                    out=s[:],
                    in_=u_ps[:],
                    func=mybir.ActivationFunctionType.Sin,
                    scale=SCALE_2PI,
                )
                nc.vector.tensor_scalar_mul(s[:], s[:], ALPHA_OVER_R)
                nc.vector.tensor_tensor(
                    x_sb[:, c, nb * UBLK : (nb + 1) * UBLK].bitcast(F32R),
                    x_sb[:, c, nb * UBLK : (nb + 1) * UBLK],
                    s[:],
                    mybir.AluOpType.add,
                )
                if nb == NUB - 1:
                    r0 = row0 + c * P
                    nc.gpsimd.dma_start(out=y_d[r0 : r0 + P, :], in_=x_sb[:, c, :])

            # super-tile layout: small first/last tiles halve pipeline
            # fill (k-loop with no tail to hide) and drain (tail with no
            # k-loop to hide)
            layout = []
            r = 0
            sizes = [256] + [SUPER] * ((rows - 512) // SUPER) + [256]
            if rows <= 512:
                sizes = [rows]
            for tok in sizes:
                layout.append((r, tok))
                r += tok
            assert r == rows

            prev = None  # (x_sb, h1_sb, row0, nch) of the previous super-tile

            for st, (row0, tok) in enumerate(layout):
                nch = tok // P
                x_sb = xpool.tile([P, nch, HIDDEN], F32)
                # column-half loads (kb-major) so the first transposes can
                # start after ~1/2 of the super-tile's data has landed
                for kb in range(2):
                    cols = slice(kb * (HIDDEN // 2), (kb + 1) * (HIDDEN // 2))
                    for c in range(nch):
                        r0 = row0 + c * P
                        nc.sync.dma_start(
                            out=x_sb[:, c, cols].bitcast(F32R),
                            in_=x_d[r0 : r0 + P, cols].bitcast(F32R),
                        )

                # mm1 k-loop of st, interleaved with the tail blocks of st-1
                ntail_prev = prev[3] * NUB if prev is not None else 0
                stride = KC // ntail_prev if ntail_prev else 0
                h1_ps = h1_psum.tile([RANK, tok], F32)
                for k in range(KC):
                    pt = pt_psum.tile([P, nch, P], F32R)
                    for c in range(nch):
                        nc.tensor.transpose(
                            pt[:, c, :],
                            x_sb[:, c, ts(k, P)].bitcast(F32R),
                            ident[:],
                        )
                    xt = xtpool.tile([P, tok], F32R)
                    nc.scalar.copy(out=xt[:], in_=pt[:])
                    nc.tensor.matmul(
                        h1_ps[:],
                        a_sb[:, k, :],
                        xt[:],
                        start=(k == 0),
                        stop=(k == KC - 1),
                    )
                    if ntail_prev and k % stride == stride - 1:
                        emit_tail_block(prev, k // stride)
                h1_sb = h1pool.tile([RANK, tok], F32R)
                nc.vector.tensor_copy(h1_sb[:], h1_ps[:])
                prev = (x_sb, h1_sb, row0, nch)

            # drain: the last super-tile's tail has no successor to hide in
            for j in range(prev[3] * NUB):
                emit_tail_block(prev, j)

    nc.compile()
    return nc


_NC_CACHE: dict[int, object] = {}


def _get_nc(rows: int = ROWS):
    nc = _NC_CACHE.get(rows)
    if nc is None:
        nc = build_nc(rows)
        _NC_CACHE[rows] = nc
    return nc


def _prep_weights(A_int8, B_int8, scale_A, scale_B):
    a_f = np.ascontiguousarray(A_int8.astype(np.float32))
    bp = np.ascontiguousarray(
        scale_A.astype(np.float32)[:, None]
        * B_int8.astype(np.float32)
        * scale_B.astype(np.float32)[None, :]
        * np.float32(1.0 / (np.pi * np.pi))
    )
    return a_f, bp


def kernel(x, A_int8, B_int8, scale_A, scale_B):
    x = np.asarray(x)
    orig_shape = x.shape
    xf = np.ascontiguousarray(x.reshape(TOTAL_ROWS, HIDDEN).astype(np.float32))
    a_f, bp = _prep_weights(
        np.asarray(A_int8), np.asarray(B_int8), np.asarray(scale_A), np.asarray(scale_B)
    )

    nc = _get_nc(ROWS)
    eye = np.eye(P, dtype=np.float32)
    in_maps = [
        {"x": xf[i * ROWS : (i + 1) * ROWS], "A": a_f, "Bp": bp, "I": eye}
        for i in range(N_CORES)
    ]
    res = run_bass_kernel_spmd(nc, in_maps, core_ids=list(range(N_CORES)))
    y = np.concatenate([r["out"] for r in res.results], axis=0)
    return y.reshape(orig_shape).astype(np.float32)



# revision 8
# speedup vs baseline: 1.2850x; 1.2850x over previous
"""PiLoraLayer TRN2 kernel: y = x + (alpha/r) * sin((2/pi) * (x @ A) @ B).

x: [4, 4096, 4096] f32; A = A_int8 * scale_A (per-col), B = B_int8 * scale_B
(per-col); rank 16 bottleneck.  alpha/r = 2.

Strategy v2 (data-parallel over 8 NeuronCores, TRANSPOSED compute space):

The v1 kernel was PE-bound: mm1 (h1 = x @ A) contracts over hidden, which
needs hidden on the partition axis, so every x tile went through a PE
transpose (512 fp32 transposes/core ~ 140us, plus 33% throttle).  v2 kills
all device transposes by shipping each core its token-shard PRE-TRANSPOSED
(hidden-major) from the host, and doing everything (mm1, mm2, sin, residual
add, output) in transposed space.

Precision plan (validated vs reference in numpy: rel err 6.1e-3 < 2e-2):
- Host ships xh = fp16(x.T / 2): fp16 halves DMA-in; /2 folds the final *2.
- mm1: h1[16, tok] = (2*A_int8 in fp16, exact).T @ xh  -> PSUM f32.
  (scale_A is folded into Bpn, so mm1 is exact except fp16(x).)
- mm2: u[128h, tok] = Bpn_chunk.T @ h1, f32r.  Bpn = -scale_A x B x scale_B
  / pi^2, so 2*pi*u = -(2/pi)*h.
- Range reduction (HW Sin LUT domain is [-pi, pi]; DVE `mod` fails the
  compiler ISA check, so): k = (u + 1.5*2^23) - 1.5*2^23 in one two-op DVE
  tensor_scalar (RNE round-to-int, bf16 exact for |k| <= 256); PE
  accumulates -k into the u PSUM bank via a bf16 negative-identity matmul,
  leaving frac in [-0.5, 0.5].
- ACT: s = fp16(Sin(frac * 2pi)) = -sin((2/pi) h).
- DVE: y_h = xh - s (all-fp16, 2x mode) = x/2 + sin((2/pi) h).
- DMA y_h out as fp16; host computes y = 2 * y_h.T.

Per-core budget (tokens=2048, hidden=4096): DMA 16 MiB in + 16 MiB out
(~91us floor at 358 GB/s); PE ~70-90us (small matmuls only); DVE 2 ops/elem
(~85us); ACT 1 op/elem (~60us).  Tokens are processed in 2 halves of 1024 so
the half-B input DMA overlaps the half-A tail.
"""

import sys

sys.path.insert(0, "/opt/trn_rl_repo")

import numpy as np

import concourse.bacc as bacc
import concourse.tile as tile
from concourse import mybir
from concourse.bass_utils import run_bass_kernel_spmd

P = 128
HIDDEN = 4096
RANK = 16
N_CORES = 8
TOTAL_ROWS = 4 * 4096
ROWS = TOTAL_ROWS // N_CORES  # 2048 tokens per core
KC = HIDDEN // P  # 32 hidden chunks
MAGIC = 12582912.0  # 1.5 * 2^23: f32 add/sub rounds to nearest integer
SCALE_2PI = 6.283185  # slightly < 2*pi: LUT arg stays inside [-pi, pi]

F32 = mybir.dt.float32
F32R = mybir.dt.float32r  # replicated fp32: 1 cycle/row on PE when N>=256
FP16 = mybir.dt.float16
BF16 = mybir.dt.bfloat16


def build_nc(tok: int = ROWS, split: int = 2):
    """Per-core program for a transposed [4096, tok] fp16 token shard."""
    assert tok % split == 0
    t_half = tok // split
    assert t_half % 512 == 0
    nb = t_half // 512  # psum-bank-wide blocks per half

    nc = bacc.Bacc(
        "TRN2",
        target_bir_lowering=False,
        debug=False,
        enable_asserts=False,
        num_devices=N_CORES,
    )
    x_d = nc.dram_tensor("xh", [HIDDEN, tok], FP16, kind="ExternalInput").ap()
    a_d = nc.dram_tensor("A2", [HIDDEN, RANK], FP16, kind="ExternalInput").ap()
    bp_d = nc.dram_tensor("Bpn", [RANK, HIDDEN], F32, kind="ExternalInput").ap()
    y_d = nc.dram_tensor("out", [HIDDEN, tok], FP16, kind="ExternalOutput").ap()

    with tile.TileContext(nc) as tc:
        with (
            tc.tile_pool(name="singles", bufs=1) as singles,
            tc.tile_pool(name="xp", bufs=2) as xpool,
            tc.tile_pool(name="h1p", bufs=1, space="PSUM") as h1_psum,
            tc.tile_pool(name="h1s", bufs=2) as h1pool,
            tc.tile_pool(name="up", bufs=3, space="PSUM") as u_psum,
            tc.tile_pool(name="tp", bufs=3) as tpool,
            tc.tile_pool(name="sp", bufs=3) as spool,
            tc.tile_pool(name="yp", bufs=3) as ypool,
        ):
            a_sb = singles.tile([P, KC, RANK], FP16)
            nc.sync.dma_start(
                out=a_sb[:], in_=a_d.rearrange("(k p) r -> p k r", p=P)
            )
            bp_sb = singles.tile([RANK, HIDDEN], F32R)
            nc.sync.dma_start(out=bp_sb[:], in_=bp_d[:, :].bitcast(F32R))
            nident_bf = singles.tile([P, P], BF16)
            nc.gpsimd.memset(nident_bf[:], 0.0)
            nc.gpsimd.affine_select(
                out=nident_bf[:],
                in_=nident_bf[:],
                compare_op=mybir.AluOpType.not_equal,
                fill=-1.0,
                base=0,
                pattern=[[-1, P]],
                channel_multiplier=1,
            )

            for hf in range(split):
                t0 = hf * t_half
                # ---- head: DMA x k-chunks in, mm1 accumulate over k ----
                x_sb = xpool.tile([P, KC, t_half], FP16)
                for k in range(KC):
                    nc.sync.dma_start(
                        out=x_sb[:, k, :],
                        in_=x_d[k * P : (k + 1) * P, t0 : t0 + t_half],
                    )
                h1_ps = h1_psum.tile([RANK, t_half], F32)
                for k in range(KC):
                    for b in range(nb):
                        nc.tensor.matmul(
                            h1_ps[:, b * 512 : (b + 1) * 512],
                            a_sb[:, k, :],
                            x_sb[:, k, b * 512 : (b + 1) * 512],
                            start=(k == 0),
                            stop=(k == KC - 1),
                        )
                h1_sb = h1pool.tile([RANK, t_half], F32R)
                nc.scalar.copy(out=h1_sb[:], in_=h1_ps[:])

                # ---- tail: per 128-hidden chunk: mm2, mod, sin, sub ----
                for c in range(KC):
                    u_ps = u_psum.tile([P, t_half], F32)
                    for b in range(nb):
                        nc.tensor.matmul(
                            u_ps[:, b * 512 : (b + 1) * 512],
                            bp_sb[:, c * P : (c + 1) * P],
                            h1_sb[:, b * 512 : (b + 1) * 512],
                            start=True,
                            stop=True,
                        )
                    kq = tpool.tile([P, t_half], BF16)
                    nc.vector.tensor_scalar(
                        kq[:],
                        u_ps[:],
                        MAGIC,
                        MAGIC,
                        mybir.AluOpType.add,
                        mybir.AluOpType.subtract,
                    )
                    for b in range(nb):
                        nc.tensor.matmul(
                            u_ps[:, b * 512 : (b + 1) * 512],
                            nident_bf[:],
                            kq[:, b * 512 : (b + 1) * 512],
                            start=False,
                            stop=True,
                            skip_group_check=True,
                        )
                    s_sb = spool.tile([P, t_half], FP16)
                    nc.scalar.activation(
                        out=s_sb[:],
                        in_=u_ps[:],
                        func=mybir.ActivationFunctionType.Sin,
                        scale=SCALE_2PI,
                    )
                    y_sb = ypool.tile([P, t_half], FP16)
                    nc.vector.tensor_tensor(
                        y_sb[:],
                        x_sb[:, c, :],
                        s_sb[:],
                        mybir.AluOpType.subtract,
                    )
                    nc.gpsimd.dma_start(
                        out=y_d[c * P : (c + 1) * P, t0 : t0 + t_half],
                        in_=y_sb[:],
                    )

    nc.compile()
    return nc


_NC_CACHE: dict[tuple, object] = {}


def _get_nc(tok: int = ROWS, split: int = 2):
    key = (tok, split)
    nc = _NC_CACHE.get(key)
    if nc is None:
        nc = build_nc(tok, split)
        _NC_CACHE[key] = nc
    return nc


def _prep_weights(A_int8, B_int8, scale_A, scale_B):
    a2 = np.ascontiguousarray((A_int8.astype(np.float32) * 2.0).astype(np.float16))
    bpn = np.ascontiguousarray(
        -scale_A.astype(np.float32)[:, None]
        * B_int8.astype(np.float32)
        * scale_B.astype(np.float32)[None, :]
        * np.float32(1.0 / (np.pi * np.pi))
    )
    return a2, bpn


def _prep_x_shard(xf, i, rows=ROWS):
    xs = xf[i * rows : (i + 1) * rows]  # [rows, 4096] f32
    return (xs.T * np.float32(0.5)).astype(np.float16)  # [4096, rows] C-contig


def kernel(x, A_int8, B_int8, scale_A, scale_B):
    x = np.asarray(x)
    orig_shape = x.shape
    xf = np.ascontiguousarray(x.reshape(TOTAL_ROWS, HIDDEN).astype(np.float32))
    a2, bpn = _prep_weights(
        np.asarray(A_int8), np.asarray(B_int8), np.asarray(scale_A), np.asarray(scale_B)
    )

    nc = _get_nc(ROWS)
    in_maps = [
        {"xh": _prep_x_shard(xf, i), "A2": a2, "Bpn": bpn} for i in range(N_CORES)
    ]
    res = run_bass_kernel_spmd(nc, in_maps, core_ids=list(range(N_CORES)))
    y = np.concatenate(
        [r["out"].astype(np.float32).T for r in res.results], axis=0
    ) * np.float32(2.0)
    return np.ascontiguousarray(y.reshape(orig_shape)).astype(np.float32)


# revision 18
# speedup vs baseline: 1.5110x; 1.1759x over previous
"""PiLoraLayer TRN2 kernel: y = x + (alpha/r) * sin((2/pi) * (x @ A) @ B).

x: [4, 4096, 4096] f32; A = A_int8 * scale_A (per-col), B = B_int8 * scale_B
(per-col); rank 16 bottleneck.  alpha/r = 2.

Strategy v2 (data-parallel over 8 NeuronCores, TRANSPOSED compute space):

The v1 kernel was PE-bound: mm1 (h1 = x @ A) contracts over hidden, which
needs hidden on the partition axis, so every x tile went through a PE
transpose (512 fp32 transposes/core ~ 140us, plus 33% throttle).  v2 kills
all device transposes by shipping each core its token-shard PRE-TRANSPOSED
(hidden-major) from the host, and doing everything (mm1, mm2, sin, residual
add, output) in transposed space.

Precision plan (validated vs reference in numpy: rel err 6.1e-3 < 2e-2):
- Host ships xh = fp16(x.T / 2): fp16 halves DMA-in; /2 folds the final *2.
- mm1: h1[16, tok] = (2*A_int8 in fp16, exact).T @ xh  -> PSUM f32.
  (scale_A is folded into Bpn, so mm1 is exact except fp16(x).)
- mm2: fp32 matmuls run 4-pass on the PE (4x columns), so u is computed as
  ONE 1-pass bf16 matmul with the hi/lo split STACKED on the contraction
  axis (PE time ~ output columns, independent of K):
    u[128h, tok] = [h1_hi; h1_lo; h1_hi] (K=48) @ [Bp_hi; Bp_hi; Bp_lo]
  which is h1 @ Bpn to ~2^-17 relative.  Bpn = -scale_A x B x scale_B
  / pi^2, so 2*pi*u = -(2/pi)*h.  Bp hi/lo are split on the host; h1 hi/lo
  on device (2 ACT copies + 1 DVE subtract on a tiny [16, tok] tile).
- Range reduction (HW Sin LUT domain is [-pi, pi]; DVE `mod` fails the
  compiler ISA check, so): k = (u + 1.5*2^23) - 1.5*2^23 in one two-op DVE
  tensor_scalar (RNE round-to-int, bf16 exact for |k| <= 256); PE
  accumulates -k into the u PSUM bank via a bf16 negative-identity matmul,
  leaving frac in [-0.5, 0.5].
- ACT: s = fp16(Sin(frac * 2pi)) = -sin((2/pi) h).
- DVE: y_h = xh - s (all-fp16, 2x mode) = x/2 + sin((2/pi) h).
- DMA y_h out as fp16; host computes y = 2 * y_h.T.

Per-core budget (tokens=2048, hidden=4096): DMA 16 MiB in + 16 MiB out
(~91us floor at 358 GB/s); PE ~70-90us (small matmuls only); DVE 2 ops/elem
(~85us); ACT 1 op/elem (~60us).  Tokens are processed in 2 halves of 1024 so
the half-B input DMA overlaps the half-A tail.
"""

import sys

sys.path.insert(0, "/opt/trn_rl_repo")

import numpy as np

import concourse.bacc as bacc
import concourse.tile as tile
from concourse import mybir
from concourse.bass_utils import run_bass_kernel_spmd

P = 128
HIDDEN = 4096
RANK = 16
N_CORES = 8
TOTAL_ROWS = 4 * 4096
ROWS = TOTAL_ROWS // N_CORES  # 2048 tokens per core
KC = HIDDEN // P  # 32 hidden chunks
MAGIC = 12582912.0  # 1.5 * 2^23: f32 add/sub rounds to nearest integer
SCALE_2PI = 6.283185  # slightly < 2*pi: LUT arg stays inside [-pi, pi]

F32 = mybir.dt.float32
F32R = mybir.dt.float32r  # replicated fp32: 1 cycle/row on PE when N>=256
FP16 = mybir.dt.float16
BF16 = mybir.dt.bfloat16


def build_nc(tok: int = ROWS, split: int = 2):
    """Per-core program for a transposed [4096, tok] fp16 token shard."""
    assert tok % split == 0
    t_half = tok // split
    assert t_half % 512 == 0
    nb = t_half // 512  # psum-bank-wide blocks per half

    nc = bacc.Bacc(
        "TRN2",
        target_bir_lowering=False,
        debug=False,
        enable_asserts=False,
        num_devices=N_CORES,
    )
    x_d = nc.dram_tensor("xh", [HIDDEN, tok], FP16, kind="ExternalInput").ap()
    a_d = nc.dram_tensor("A2", [HIDDEN, RANK], FP16, kind="ExternalInput").ap()
    bp_d = nc.dram_tensor("Bps", [6 * RANK, HIDDEN], BF16, kind="ExternalInput").ap()
    y_d = nc.dram_tensor("out", [HIDDEN, tok], FP16, kind="ExternalOutput").ap()

    with tile.TileContext(nc) as tc:
        with (
            tc.tile_pool(name="singles", bufs=1) as singles,
            tc.tile_pool(name="xp", bufs=2) as xpool,
            tc.tile_pool(name="h1p", bufs=1, space="PSUM") as h1_psum,
            tc.tile_pool(name="h1s", bufs=2) as h1pool,
            tc.tile_pool(name="up", bufs=3, space="PSUM") as u_psum,
            tc.tile_pool(name="tp", bufs=3) as tpool,
            tc.tile_pool(name="sp", bufs=3) as spool,
            tc.tile_pool(name="yp", bufs=3) as ypool,
        ):
            a_sb = singles.tile([P, KC, RANK], FP16)
            nc.sync.dma_start(
                out=a_sb[:], in_=a_d.rearrange("(k p) r -> p k r", p=P)
            )
            bp_sb = singles.tile([6 * RANK, HIDDEN], BF16)
            nc.sync.dma_start(out=bp_sb[:], in_=bp_d[:, :])
            nident_bf = singles.tile([P, P], BF16)
            nc.gpsimd.memset(nident_bf[:], 0.0)
            nc.gpsimd.affine_select(
                out=nident_bf[:],
                in_=nident_bf[:],
                compare_op=mybir.AluOpType.not_equal,
                fill=-1.0,
                base=0,
                pattern=[[-1, P]],
                channel_multiplier=1,
            )

            for hf in range(split):
                t0 = hf * t_half
                # ---- head: DMA x k-chunks in, mm1 accumulate over k ----
                x_sb = xpool.tile([P, KC, t_half], FP16)
                for k in range(KC):
                    nc.sync.dma_start(
                        out=x_sb[:, k, :],
                        in_=x_d[k * P : (k + 1) * P, t0 : t0 + t_half],
                    )
                h1_ps = h1_psum.tile([RANK, t_half], F32)
                for k in range(KC):
                    for b in range(nb):
                        nc.tensor.matmul(
                            h1_ps[:, b * 512 : (b + 1) * 512],
                            a_sb[:, k, :],
                            x_sb[:, k, b * 512 : (b + 1) * 512],
                            start=(k == 0),
                            stop=(k == KC - 1),
                        )
                # h1 hi/lo bf16 split, stacked [h1_hi; _; h1_lo; _; h1_hi; _]
                # at 32-aligned starts, for the single K=96 bf16 mm2
                h1_sb = h1pool.tile([6 * RANK, t_half], BF16)
                nc.gpsimd.memset(h1_sb[:], 0.0)
                nc.scalar.copy(out=h1_sb[0:RANK, :], in_=h1_ps[:])
                nc.vector.tensor_tensor(
                    h1_sb[2 * RANK : 3 * RANK, :],
                    h1_ps[:],
                    h1_sb[0:RANK, :],
                    mybir.AluOpType.subtract,
                )
                nc.scalar.copy(out=h1_sb[4 * RANK : 5 * RANK, :], in_=h1_ps[:])

                # ---- tail: per 128-hidden chunk: mm2, mod, sin, sub ----
                for c in range(KC):
                    u_ps = u_psum.tile([P, t_half], F32)
                    for b in range(nb):
                        nc.tensor.matmul(
                            u_ps[:, b * 512 : (b + 1) * 512],
                            bp_sb[:, c * P : (c + 1) * P],
                            h1_sb[:, b * 512 : (b + 1) * 512],
                            start=True,
                            stop=True,
                        )
                    kq = tpool.tile([P, t_half], BF16)
                    nc.vector.tensor_scalar(
                        kq[:],
                        u_ps[:],
                        MAGIC,
                        MAGIC,
                        mybir.AluOpType.add,
                        mybir.AluOpType.subtract,
                    )
                    for b in range(nb):
                        nc.tensor.matmul(
                            u_ps[:, b * 512 : (b + 1) * 512],
                            nident_bf[:],
                            kq[:, b * 512 : (b + 1) * 512],
                            start=False,
                            stop=True,
                            skip_group_check=True,
                        )
                    s_sb = spool.tile([P, t_half], FP16)
                    nc.scalar.activation(
                        out=s_sb[:],
                        in_=u_ps[:],
                        func=mybir.ActivationFunctionType.Sin,
                        scale=SCALE_2PI,
                    )
                    y_sb = ypool.tile([P, t_half], FP16)
                    nc.vector.tensor_tensor(
                        y_sb[:],
                        x_sb[:, c, :],
                        s_sb[:],
                        mybir.AluOpType.subtract,
                    )
                    nc.gpsimd.dma_start(
                        out=y_d[c * P : (c + 1) * P, t0 : t0 + t_half],
                        in_=y_sb[:],
                    )

    nc.compile()
    return nc


_NC_CACHE: dict[tuple, object] = {}


def _get_nc(tok: int = ROWS, split: int = 2):
    key = (tok, split)
    nc = _NC_CACHE.get(key)
    if nc is None:
        nc = build_nc(tok, split)
        _NC_CACHE[key] = nc
    return nc


def _prep_weights(A_int8, B_int8, scale_A, scale_B):
    import ml_dtypes

    a2 = np.ascontiguousarray((A_int8.astype(np.float32) * 2.0).astype(np.float16))
    bpn = (
        -scale_A.astype(np.float32)[:, None]
        * B_int8.astype(np.float32)
        * scale_B.astype(np.float32)[None, :]
        * np.float32(1.0 / (np.pi * np.pi))
    )
    bp_hi = bpn.astype(ml_dtypes.bfloat16)
    bp_lo = (bpn - bp_hi.astype(np.float32)).astype(ml_dtypes.bfloat16)
    # engines address partitions at 32-aligned starts only, so each 16-row
    # block sits at a 32-row offset; the zero pad rows kill the pad terms.
    z = np.zeros_like(bp_hi)
    bps = np.ascontiguousarray(
        np.concatenate([bp_hi, z, bp_hi, z, bp_lo, z], axis=0)
    )
    return a2, bps


def _prep_x_shard(xf, i, rows=ROWS):
    xs = xf[i * rows : (i + 1) * rows]  # [rows, 4096] f32
    return (xs.T * np.float32(0.5)).astype(np.float16)  # [4096, rows] C-contig


def kernel(x, A_int8, B_int8, scale_A, scale_B):
    x = np.asarray(x)
    orig_shape = x.shape
    xf = np.ascontiguousarray(x.reshape(TOTAL_ROWS, HIDDEN).astype(np.float32))
    a2, bps = _prep_weights(
        np.asarray(A_int8), np.asarray(B_int8), np.asarray(scale_A), np.asarray(scale_B)
    )

    nc = _get_nc(ROWS)
    in_maps = [
        {"xh": _prep_x_shard(xf, i), "A2": a2, "Bps": bps} for i in range(N_CORES)
    ]
    res = run_bass_kernel_spmd(nc, in_maps, core_ids=list(range(N_CORES)))
    y = np.concatenate(
        [r["out"].astype(np.float32).T for r in res.results], axis=0
    ) * np.float32(2.0)
    return np.ascontiguousarray(y.reshape(orig_shape)).astype(np.float32)
